# revision 28
# baseline (speedup 1.0000x reference)
"""Trainium2 Bass kernel for the spectral-gating network (nn_DAPSO).

Model (B=4, C=256, H=W=256):
  - channels 0:128   : y_h = irfft(Gh * rfft(x, axis=H))   (per-channel gate)
  - channels 128:256 : y_w = irfft(Gw * rfft(x, axis=W))
  - gates Gh/Gw from tiny MLPs (computed on device)
  - channel attention: s = sigmoid(dw(gelu(W1 @ mean_hw(y) + b)))  -> y *= s
  - y2 = gelu(BN(lc_w @ y));  out = x + y2

Key algorithmic mapping: irfft(G*rfft(x)) along an axis of length N equals
T^T diag(ghat) T x with T the orthonormal real DFT basis (cos/sin rows), so
both branches become dense TensorE matmuls (no FFT).

Sharding: 8 cores = 4 batches x 2 w-halves. Each core computes BOTH branch
outputs for its (batch, w-half) spatial region: the H-branch needs only its
w-columns; the W-branch contracts the full W axis (its forward transform is
duplicated between the pair of cores). The only cross-core communication is
a 1KB AllReduce of the pooled channel means (one per branch).

Fully SBUF-resident: the inverse transforms use the gated spectrum slice as
the *stationary* (lhsT) matmul operand, so branch outputs land channel-major
([c, spatial]) directly in SBUF (yT_h, yT_w; 8.4MB bf16 each) and the 1x1
conv consumes them without any DRAM round-trip.

Channel attention pooling comes for free from the forward spectra:
  sum_h y_h = ghat_h[0] * sqrt(N) * u_h[k=0]        (DC row strip)
  sum_{w in Ws} y_w = sum_k sigw[k] ghat_w[k,c] S[k,c],  S = sum_h u_w

Per-core layouts (host-prepped):
  xh   (256,128,128) bf16  [h, w, c]     HC-branch input slice
  xw   (256,256,128) bf16  [w, h, c]     WC-branch input (full w)
  xres (256,256,128) bf16  [c, h, w]     residual slice
  out  (256,256,128) bf16  [c, h, w]     (host casts back to f32)
"""
import sys
import os

sys.path.insert(0, "/opt/trn_rl_repo")

import numpy as np
import ml_dtypes

import concourse.bacc as bacc
import concourse.mybir as mybir
import concourse.tile as tile
from concourse import bass_utils

F32 = mybir.dt.float32
BF16 = mybir.dt.bfloat16
AF = mybir.ActivationFunctionType
ALU = mybir.AluOpType

N = 256          # H = W
C2 = 128         # channels per branch
B = 4
NCORES = 8
WS = 128         # per-core w-slice width

_BF16_NP = ml_dtypes.bfloat16


def _dft_basis():
    """Orthonormal real DFT basis T (N, N): y = T^T diag(ghat) T x == irfft(G*rfft(x))."""
    n = np.arange(N)
    k = np.arange(1, N // 2)
    T = np.zeros((N, N), np.float64)
    T[0, :] = 1.0 / np.sqrt(N)
    T[1:N // 2, :] = np.sqrt(2.0 / N) * np.cos(2 * np.pi * k[:, None] * n[None, :] / N)
    T[N // 2, :] = (1.0 / np.sqrt(N)) * ((-1.0) ** n)
    T[N // 2 + 1:, :] = np.sqrt(2.0 / N) * np.sin(2 * np.pi * k[:, None] * n[None, :] / N)
    return T.astype(np.float32)


def _part_major(a):
    """(256, ...) -> (128, 2, ...) partition-major layout."""
    a = np.asarray(a)
    return np.ascontiguousarray(a.reshape(2, 128, *a.shape[1:]).transpose(
        (1, 0) + tuple(range(2, a.ndim + 1))))


_MLPS = ("ah", "bc1", "aw", "bc2")


def _build():
    nc = bacc.Bacc("TRN2", target_bir_lowering=False, num_devices=NCORES)

    # ---------------- I/O declarations ----------------
    xh_d = nc.dram_tensor("xh", [256, 128, 128], BF16, kind="ExternalInput")
    xw_d = nc.dram_tensor("xw", [256, 256, 128], BF16, kind="ExternalInput")
    xres_d = nc.dram_tensor("xres", [256, 256, 128], BF16, kind="ExternalInput")
    tfwd_d = nc.dram_tensor("tfwd", [128, 2, 256], BF16, kind="ExternalInput")
    tinv_d = nc.dram_tensor("tinv", [128, 2, 256], BF16, kind="ExternalInput")
    tinvw_d = nc.dram_tensor("tinvw", [128, 2, 128], BF16, kind="ExternalInput")
    sigw_d = nc.dram_tensor("sigw", [128, 2], F32, kind="ExternalInput")
    omega_d = nc.dram_tensor("omega", [1, 129], F32, kind="ExternalInput")
    lam_d = nc.dram_tensor("lam", [1, 128], F32, kind="ExternalInput")
    mlp_d = {}
    for m in _MLPS:
        mlp_d[m] = dict(
            w1t=nc.dram_tensor(f"{m}_w1t", [1, 64], F32, kind="ExternalInput"),
            b1=nc.dram_tensor(f"{m}_b1v", [64, 1], F32, kind="ExternalInput"),
            w2t=nc.dram_tensor(f"{m}_w2t", [64, 64], F32, kind="ExternalInput"),
            b2=nc.dram_tensor(f"{m}_b2v", [64, 1], F32, kind="ExternalInput"),
            w3t=nc.dram_tensor(f"{m}_w3t", [64, 8], F32, kind="ExternalInput"),
            b3=nc.dram_tensor(f"{m}_b3v", [8, 1], F32, kind="ExternalInput"),
        )
    caw1t_d = nc.dram_tensor("caw1t", [128, 2, 256], F32, kind="ExternalInput")
    cab1_d = nc.dram_tensor("cab1", [128, 2], F32, kind="ExternalInput")
    dwc_d = nc.dram_tensor("dwc", [128, 2], F32, kind="ExternalInput")
    dwb_d = nc.dram_tensor("dwb", [128, 2], F32, kind="ExternalInput")
    lcwt_d = nc.dram_tensor("lcwt", [128, 2, 256], F32, kind="ExternalInput")
    bng_d = nc.dram_tensor("bng", [128, 2], F32, kind="ExternalInput")
    bnb_d = nc.dram_tensor("bnb", [128, 2], F32, kind="ExternalInput")
    bnm_d = nc.dram_tensor("bnm", [128, 2], F32, kind="ExternalInput")
    bnv_d = nc.dram_tensor("bnv", [128, 2], F32, kind="ExternalInput")

    out_d = nc.dram_tensor("out", [256, 256, 128], BF16, kind="ExternalOutput")

    arh_in = nc.dram_tensor("arh_in", [128, 1], F32)
    arh_out = nc.dram_tensor("arh_out", [128, 1], F32)
    arw_in = nc.dram_tensor("arw_in", [128, 1], F32)
    arw_out = nc.dram_tensor("arw_out", [128, 1], F32)

    with tile.TileContext(nc) as tc:
        with tc.tile_pool(name="consts", bufs=1) as consts, \
             tc.tile_pool(name="yres", bufs=1) as yres, \
             tc.tile_pool(name="xin", bufs=2) as xin, \
             tc.tile_pool(name="uch", bufs=2) as uch, \
             tc.tile_pool(name="crhs", bufs=2) as crhs, \
             tc.tile_pool(name="outp", bufs=2) as outp, \
             tc.tile_pool(name="gsb", bufs=1) as gsb, \
             tc.tile_pool(name="ps", bufs=1, space="PSUM") as ps:

            # ---------------- const loads ----------------
            # MLP weights FIRST: the gate matmuls lead the in-order Tensor
            # stream, so their weights must not queue behind bulk loads.
            mlp_t = {}
            for m in _MLPS:
                d = mlp_d[m]
                mlp_t[m] = {}
                for nm, shp in (("w1t", [1, 64]), ("b1", [64, 1]),
                                ("w2t", [64, 64]), ("b2", [64, 1]),
                                ("w3t", [64, 8]), ("b3", [8, 1])):
                    t = consts.tile(shp, F32, tag=f"{m}_{nm}")
                    nc.sync.dma_start(out=t, in_=d[nm][:])
                    mlp_t[m][nm] = t
            omega_t = consts.tile([1, 129], F32, tag="omega")
            nc.sync.dma_start(out=omega_t, in_=omega_d[:])
            lam_t = consts.tile([1, 128], F32, tag="lam")
            nc.sync.dma_start(out=lam_t, in_=lam_d[:])
            tfwd_t = consts.tile([128, 2, 256], BF16, tag="tfwd")
            nc.sync.dma_start(out=tfwd_t, in_=tfwd_d[:])
            tinv_t = consts.tile([128, 2, 256], BF16, tag="tinv")
            nc.sync.dma_start(out=tinv_t, in_=tinv_d[:])
            tinvw_t = consts.tile([128, 2, 128], BF16, tag="tinvw")
            nc.sync.dma_start(out=tinvw_t, in_=tinvw_d[:])
            sigw_t = consts.tile([128, 2], F32, tag="sigw")
            nc.sync.dma_start(out=sigw_t, in_=sigw_d[:])
            caw1t_t = consts.tile([128, 2, 256], F32, tag="caw1t")
            nc.sync.dma_start(out=caw1t_t, in_=caw1t_d[:])
            lcwt_t = consts.tile([128, 2, 256], F32, tag="lcwt")
            nc.sync.dma_start(out=lcwt_t, in_=lcwt_d[:])
            vec_t = {}
            for nm, d in (("cab1", cab1_d), ("dwc", dwc_d), ("dwb", dwb_d),
                          ("bng", bng_d), ("bnb", bnb_d), ("bnm", bnm_d), ("bnv", bnv_d)):
                vt = consts.tile([128, 2], F32, tag=f"v_{nm}")
                nc.sync.dma_start(out=vt, in_=d[:])
                vec_t[nm] = vt
            one1_t = consts.tile([1, 1], F32, tag="one1")
            nc.vector.memset(one1_t, 1.0)

            # ---------------- gate MLPs (tiny), stage-batched ----------------
            # All four heads advance layer-by-layer so each activation table
            # (Gelu) loads once per sweep instead of thrashing per-head.
            heads = (("aw", omega_t, 129), ("bc2", lam_t, 128),
                     ("ah", omega_t, 129), ("bc1", lam_t, 128))
            ptags = ("A00", "B0", "A01", "B1")
            p1 = {}
            for i, (m, xv, nk) in enumerate(heads):
                p = ps.tile([64, nk], F32, tag=ptags[i])
                nc.tensor.matmul(p, lhsT=mlp_t[m]["w1t"], rhs=xv, start=True, stop=True)
                p1[m] = p
            h1 = {}
            for i, (m, xv, nk) in enumerate(heads):
                h = gsb.tile([64, nk], F32, tag=f"m_h1_{i}")
                nc.scalar.activation(h, p1[m], AF.Gelu, bias=mlp_t[m]["b1"])
                h1[m] = h
            p2 = {}
            for i, (m, xv, nk) in enumerate(heads):
                p = ps.tile([64, nk], F32, tag=ptags[i])
                nc.tensor.matmul(p, lhsT=mlp_t[m]["w2t"], rhs=h1[m], start=True, stop=True)
                p2[m] = p
            h2 = {}
            for i, (m, xv, nk) in enumerate(heads):
                h = gsb.tile([64, nk], F32, tag=f"m_h1_{i}")
                nc.scalar.activation(h, p2[m], AF.Gelu, bias=mlp_t[m]["b2"])
                h2[m] = h
            at = {}
            for i, (m, xv, nk) in enumerate(heads):
                p = ps.tile([8, nk], F32, tag=ptags[i])
                nc.tensor.matmul(p, lhsT=mlp_t[m]["w3t"], rhs=h2[m], start=True, stop=True)
                a = gsb.tile([8, nk], F32, tag=f"m_at{i}")
                nc.scalar.activation(a, p, AF.Identity, bias=mlp_t[m]["b3"])
                at[m] = a

            ghh = consts.tile([128, 2, 128], F32, tag="ghh")
            ghw = consts.tile([128, 2, 128], F32, tag="ghw")
            ghhb = consts.tile([128, 2, 128], BF16, tag="ghhb")
            ghwb = consts.tile([128, 2, 128], BF16, tag="ghwb")
            # transposed DC-row gate column for pool_h: G^T[:, 0] pre-softplus
            g0c_ps = ps.tile([128, 1], F32, tag="B2")
            nc.tensor.matmul(g0c_ps, lhsT=at["bc1"], rhs=at["ah"][:, 0:1],
                             start=True, stop=True)
            g0col = gsb.tile([128, 1], F32, tag="g0col")
            nc.scalar.activation(g0col, g0c_ps, AF.Sigmoid, scale=-1.0)
            nc.scalar.activation(g0col, g0col, AF.Ln)
            nc.vector.tensor_scalar_mul(g0col, g0col, float(-(8.0 ** -0.5)))

            gp = {}
            gtags = {("h", 0): "A00", ("h", 1): "A01", ("w", 0): "B0", ("w", 1): "B1"}
            for (am, bm, nmk) in (("aw", "bc2", "w"), ("ah", "bc1", "h")):
                g0 = ps.tile([128, 128], F32, tag=gtags[(nmk, 0)])
                nc.tensor.matmul(g0, lhsT=at[am][:, 0:128], rhs=at[bm], start=True, stop=True)
                gn = ps.tile([1, 128], F32, tag=gtags[(nmk, 1)])
                nc.tensor.matmul(gn, lhsT=at[am][:, 128:129], rhs=at[bm], start=True, stop=True)
                gp[(nmk, 0)] = g0
                gp[(nmk, 1)] = gn
            # softplus(z) = -ln(sigmoid(-z)); the -1 and 1/sqrt(R) fold into
            # the final scale pass.
            keys = list(gp.keys())
            sp = {}
            for i, key in enumerate(keys):
                npart = 128 if key[1] == 0 else 1
                sg = gsb.tile([128, 128], F32, tag=f"sp_sg{i}")
                nc.scalar.activation(sg[:npart, :], gp[key], AF.Sigmoid, scale=-1.0)
                sp[key] = sg
            for key in keys:
                gh = ghh if key[0] == "h" else ghw
                if key[1] == 0:
                    nc.scalar.activation(gh[:, 0, :], sp[key][:128, :], AF.Ln)
                else:
                    nc.scalar.activation(sp[key][0:1, :], sp[key][0:1, :], AF.Ln)
            for key in keys:
                gh = ghh if key[0] == "h" else ghw
                if key[1] == 1:
                    # rows 128+j of ghat equal G[j]: copy the aligned block,
                    # then overwrite row 0 with the Nyquist G[128].
                    nc.vector.tensor_copy(gh[:, 1, :], gh[:, 0, :])
                    nc.vector.tensor_copy(gh[0:1, 1, :], sp[key][0:1, :])
            for gh, ghb in ((ghh, ghhb), (ghw, ghwb)):
                nc.vector.tensor_scalar_mul(gh[:, :, :], gh[:, :, :], float(-(8.0 ** -0.5)))
                nc.vector.tensor_copy(ghb, gh)

            # ---------------- BN prep ----------------
            bninv = consts.tile([128, 2], F32, tag="bninv")
            nc.vector.tensor_scalar_add(bninv, vec_t["bnv"], 1e-5)
            nc.scalar.activation(bninv, bninv, AF.Sqrt)
            nc.vector.reciprocal(bninv, bninv)
            nc.vector.tensor_tensor(out=bninv, in0=vec_t["bng"], in1=bninv, op=ALU.mult)
            bnbeff = consts.tile([128, 2], F32, tag="bnbeff")
            nc.vector.tensor_tensor(out=bnbeff, in0=vec_t["bnm"], in1=bninv, op=ALU.mult)
            nc.vector.tensor_tensor(out=bnbeff, in0=vec_t["bnb"], in1=bnbeff, op=ALU.subtract)

            # branch outputs, channel-major, SBUF-resident
            yT_h = yres.tile([128, 256, 128], BF16, tag="yTh")   # [c, h, w]
            yT_w = yres.tile([128, 256, 128], BF16, tag="yTw")   # [c, h, w]

            # ---------------- early pool_h from xres (already c-major) -------
            # pool_h[c] = Gh[0, c] * sum_{h,w} x[c, h, w]; AllReduce #1 (65us
            # latency) is issued ~40us in and hides under branch compute.
            xacc = gsb.tile([128, 8], F32, tag="xacc")
            nc.vector.memset(xacc, 0.0)
            for hc in range(0, 256, 8):
                xt = crhs.tile([128, 8, 128], BF16, tag="xpre")
                nc.scalar.dma_start(out=xt, in_=xres_d[0:128, hc:hc + 8, :])
                red = gsb.tile([128, 8], F32, tag="xred")
                nc.vector.tensor_reduce(out=red, in_=xt,
                                        axis=mybir.AxisListType.X, op=ALU.add)
                nc.vector.tensor_tensor(out=xacc, in0=xacc, in1=red, op=ALU.add)
            xcol = gsb.tile([128, 1], F32, tag="xcol")
            nc.vector.tensor_reduce(out=xcol, in_=xacc,
                                    axis=mybir.AxisListType.X, op=ALU.add)
            poolh_sb = gsb.tile([128, 1], F32, tag="poolh")
            nc.vector.tensor_tensor(out=poolh_sb, in0=g0col, in1=xcol, op=ALU.mult)
            nc.sync.dma_start(out=arh_in[:], in_=poolh_sb)
            nc.gpsimd.collective_compute(
                "AllReduce", ALU.add,
                replica_groups=[[0, 1], [2, 3], [4, 5], [6, 7]],
                ins=[arh_in[:]], outs=[arh_out[:]])

            wacc = gsb.tile([128, 32], F32, tag="wacc")

            # ---------------- branches, interleaved ----------------
            # WC chunks are Vector-heavy (gate-mults), HC chunks are
            # scatter-heavy (Scalar/GpSimd); interleaving (W,W,H) lets idle
            # engines absorb each other's load. WC finishes 2/3 in so its
            # AllReduce still hides.
            def wc_chunk(h0):
                xw_t = []
                for wt in (0, 1):
                    xt = xin.tile([128, 8, 128], BF16, tag=f"xb{wt}")
                    nc.sync.dma_start(out=xt, in_=xw_d[wt * 128:(wt + 1) * 128,
                                                      h0:h0 + 8, :])
                    xw_t.append(xt)
                ug = {}
                for kt in (0, 1):
                    for hf in (0, 1):
                        pk = ps.tile([128, 4, 128], F32, tag=f"A{kt}{hf}")
                        for wt in (0, 1):
                            nc.tensor.matmul(pk,
                                             lhsT=tfwd_t[:, wt, kt * 128:(kt + 1) * 128],
                                             rhs=xw_t[wt][:, hf * 4:hf * 4 + 4, :],
                                             start=(wt == 0), stop=(wt == 1))
                        u = uch.tile([128, 4, 128], BF16, tag=f"ug{kt}{hf}")
                        nc.vector.tensor_tensor(
                            out=u, in0=pk,
                            in1=ghwb[:, kt, :].unsqueeze(1).broadcast_to([128, 4, 128]),
                            op=ALU.mult)
                        ug[(kt, hf)] = u
                for q in (0, 1):
                    py = ps.tile([128, 4, 128], F32, tag=f"B{q}")
                    for j in range(4):
                        hi = q * 4 + j
                        for kt in (0, 1):
                            nc.tensor.matmul(py[:, j, :],
                                             lhsT=ug[(kt, hi // 4)][:, hi % 4, :],
                                             rhs=tinvw_t[:, kt, :],
                                             start=(kt == 0), stop=(kt == 1))
                    nc.scalar.activation(yT_w[:, h0 + q * 4:h0 + q * 4 + 4, :], py,
                                         AF.Copy)

            def hc_chunk(w0):
                xh_t = []
                for ht in (0, 1):
                    xt = xin.tile([128, WCH, 128], BF16, tag=f"xa{ht}")
                    nc.sync.dma_start(out=xt, in_=xh_d[ht * 128:(ht + 1) * 128,
                                                      w0:w0 + WCH, :])
                    xh_t.append(xt)
                ug = {}
                for kt in (0, 1):
                    for hf in (0, 1):
                        pk = ps.tile([128, 4, 128], F32, tag=f"A{kt}{hf}")
                        for ht in (0, 1):
                            nc.tensor.matmul(pk,
                                             lhsT=tfwd_t[:, ht, kt * 128:(kt + 1) * 128],
                                             rhs=xh_t[ht][:, hf * 4:hf * 4 + 4, :],
                                             start=(ht == 0), stop=(ht == 1))
                        u = uch.tile([128, 4, 128], BF16, tag=f"uh{kt}{hf}")
                        nc.vector.tensor_tensor(
                            out=u, in0=pk,
                            in1=ghhb[:, kt, :].unsqueeze(1).broadcast_to([128, 4, 128]),
                            op=ALU.mult)
                        ug[(kt, hf)] = u
                for wi in range(WCH):
                    py = ps.tile([128, 256], F32, tag=f"B{2 + wi % 2}")
                    for kt in (0, 1):
                        nc.tensor.matmul(py, lhsT=ug[(kt, wi // 4)][:, wi % 4, :],
                                         rhs=tinv_t[:, kt, :],
                                         start=(kt == 0), stop=(kt == 1))
                    hst = uch.tile([128, 256], BF16, tag=f"hst{wi % 2}")
                    nc.scalar.activation(hst, py, AF.Copy)
                    if wi % 2 == 0:
                        nc.gpsimd.tensor_copy(yT_h[:, :, w0 + wi], hst)
                    else:
                        nc.vector.tensor_copy(yT_h[:, :, w0 + wi], hst)

            WCH = 8
            wc_i, hc_i = 0, 0
            for step in range(48):
                if step % 3 < 2 and wc_i < 32:
                    wc_chunk(wc_i * 8)
                    wc_i += 1
                elif hc_i < 16:
                    hc_chunk(hc_i * 8)
                    hc_i += 1

            # pool_w[c] = sum_{h, local w} y_w[c, h, w]  -> AllReduce #2
            # (y_w is the pooled quantity; reduce it in slabs off the
            # per-chunk critical path)
            for sl in range(8):
                red = gsb.tile([128, 32], F32, tag="wred")
                nc.vector.tensor_reduce(out=red, in_=yT_w[:, sl * 32:(sl + 1) * 32, :],
                                        axis=mybir.AxisListType.X, op=ALU.add)
                if sl == 0:
                    nc.vector.tensor_copy(wacc, red)
                else:
                    nc.vector.tensor_tensor(out=wacc, in0=wacc, in1=red, op=ALU.add)
            poolw_sb = gsb.tile([128, 1], F32, tag="poolw")
            nc.vector.tensor_reduce(out=poolw_sb, in_=wacc,
                                    axis=mybir.AxisListType.X, op=ALU.add)
            nc.sync.dma_start(out=arw_in[:], in_=poolw_sb)
            nc.gpsimd.collective_compute(
                "AllReduce", ALU.add,
                replica_groups=[[0, 1], [2, 3], [4, 5], [6, 7]],
                ins=[arw_in[:]], outs=[arw_out[:]])

            p_sb = []
            for ct, aro in ((0, arh_out), (1, arw_out)):
                pt = gsb.tile([128, 1], F32, tag=f"p_ar{ct}")
                nc.gpsimd.dma_start(out=pt, in_=aro[:])
                p_sb.append(pt)

            # ---------------- channel attention -> folded conv weights ----------------
            q_sb = []
            for ot in (0, 1):
                q_ps = ps.tile([128, 1], F32, tag=f"B{ot}")
                for ct in (0, 1):
                    nc.tensor.matmul(q_ps, lhsT=caw1t_t[:, ct, ot * 128:(ot + 1) * 128],
                                     rhs=p_sb[ct], start=(ct == 0), stop=(ct == 1))
                qt = gsb.tile([128, 1], F32, tag=f"q{ot}")
                nc.scalar.activation(qt, q_ps, AF.Gelu, bias=vec_t["cab1"][:, ot:ot + 1])
                nc.vector.tensor_tensor(out=qt, in0=qt, in1=vec_t["dwc"][:, ot:ot + 1],
                                        op=ALU.mult)
                q_sb.append(qt)
            s_sb = []
            for ot in (0, 1):
                s_t = gsb.tile([128, 1], F32, tag=f"s{ot}")
                nc.scalar.activation(s_t, q_sb[ot], AF.Sigmoid, bias=vec_t["dwb"][:, ot:ot + 1])
                s_sb.append(s_t)
            wsc = consts.tile([128, 2, 256], BF16, tag="wsc")
            for ct in (0, 1):
                nc.vector.tensor_scalar_mul(wsc[:, ct, :], lcwt_t[:, ct, :], s_sb[ct])

            # ---------------- conv 1x1 + BN + GELU + residual add ----------------
            HCH = 8
            for h0 in range(0, 256, HCH):
                xts = []
                for ot in (0, 1):
                    xt = crhs.tile([128, HCH, 128], BF16, tag=f"xr{ot}")
                    nc.scalar.dma_start(out=xt, in_=xres_d[ot * 128:(ot + 1) * 128,
                                                           h0:h0 + HCH, :])
                    xts.append(xt)
                for ot in (0, 1):
                    gstg = outp.tile([128, HCH, 128], BF16, tag=f"gstg{ot}")
                    for sl in (0, 4):
                        rh = yT_h[:, h0 + sl:h0 + sl + 4, :]
                        rw = yT_w[:, h0 + sl:h0 + sl + 4, :]
                        po = ps.tile([128, 4, 128], F32, tag=f"A{ot}{sl // 4}")
                        nc.tensor.matmul(po, lhsT=wsc[:, 0, ot * 128:(ot + 1) * 128],
                                         rhs=rh, start=True, stop=False)
                        nc.tensor.matmul(po, lhsT=wsc[:, 1, ot * 128:(ot + 1) * 128],
                                         rhs=rw, start=False, stop=True)
                        nc.scalar.activation(gstg[:, sl:sl + 4, :], po, AF.Gelu,
                                             bias=bnbeff[:, ot:ot + 1],
                                             scale=bninv[:, ot:ot + 1])
                    nc.vector.tensor_tensor(out=gstg, in0=gstg, in1=xts[ot],
                                            op=ALU.add)
                    nc.sync.dma_start(out=out_d[ot * 128:(ot + 1) * 128, h0:h0 + HCH, :],
                                        in_=gstg)

    nc.compile()
    return nc


_NC_CACHE = None


def _get_nc():
    global _NC_CACHE
    if _NC_CACHE is None:
        _NC_CACHE = _build()
    return _NC_CACHE


def _host_consts(inputs, core):
    """Per-core constant inputs (everything except the x shards)."""
    s = core % 2
    wlo = WS * s
    T = _dft_basis()
    d = {}
    d["tfwd"] = _part_major(np.ascontiguousarray(T.T)).astype(_BF16_NP)
    d["tinv"] = _part_major(T).astype(_BF16_NP)
    d["tinvw"] = _part_major(np.ascontiguousarray(T[:, wlo:wlo + WS])).astype(_BF16_NP)
    d["sigw"] = _part_major(T[:, wlo:wlo + WS].sum(axis=1)).astype(np.float32)
    d["omega"] = (np.arange(129, dtype=np.float32) / 128.0 - 1.0).reshape(1, 129)
    d["lam"] = np.linspace(-1.0, 1.0, 128, dtype=np.float32).reshape(1, 128)
    for m in _MLPS:
        d[f"{m}_w1t"] = np.ascontiguousarray(inputs[f"{m}_w1"].T).astype(np.float32)
        d[f"{m}_b1v"] = inputs[f"{m}_b1"].reshape(64, 1).astype(np.float32)
        d[f"{m}_w2t"] = np.ascontiguousarray(inputs[f"{m}_w2"].T).astype(np.float32)
        d[f"{m}_b2v"] = inputs[f"{m}_b2"].reshape(64, 1).astype(np.float32)
        d[f"{m}_w3t"] = np.ascontiguousarray(inputs[f"{m}_w3"].T).astype(np.float32)
        d[f"{m}_b3v"] = inputs[f"{m}_b3"].reshape(8, 1).astype(np.float32)
    d["caw1t"] = _part_major(np.ascontiguousarray(inputs["ca_w1"].T) / 65536.0).astype(np.float32)
    d["cab1"] = _part_major(inputs["ca_b1"]).astype(np.float32)
    d["dwc"] = _part_major(np.ascontiguousarray(inputs["ca_dw"][:, 1, 1])).astype(np.float32)
    d["dwb"] = _part_major(inputs["ca_db"]).astype(np.float32)
    d["lcwt"] = _part_major(np.ascontiguousarray(inputs["lc_w"].T)).astype(np.float32)
    d["bng"] = _part_major(inputs["bn_g"]).astype(np.float32)
    d["bnb"] = _part_major(inputs["bn_b"]).astype(np.float32)
    d["bnm"] = _part_major(inputs["bn_m"]).astype(np.float32)
    d["bnv"] = _part_major(inputs["bn_v"]).astype(np.float32)
    return d


def kernel(**inputs):
    x = np.asarray(inputs["x"], np.float32)
    nc = _get_nc()

    in_maps = []
    for core in range(NCORES):
        b, s = core // 2, core % 2
        wlo = WS * s
        m = _host_consts(inputs, core)
        m["xh"] = np.ascontiguousarray(
            x[b, :C2, :, wlo:wlo + WS].transpose(1, 2, 0)).astype(_BF16_NP)
        m["xw"] = np.ascontiguousarray(
            x[b, C2:, :, :].transpose(2, 1, 0)).astype(_BF16_NP)
        m["xres"] = np.ascontiguousarray(x[b, :, :, wlo:wlo + WS]).astype(_BF16_NP)
        in_maps.append(m)

    trace = os.environ.get("BASS_KERNEL_TRACE", "0") == "1"
    res = bass_utils.run_bass_kernel_spmd(
        nc, in_maps, core_ids=list(range(NCORES)),
        trace=trace, trace_cores=list(range(NCORES)) if trace else None,
        stitch_traces=False)
    if trace and res.exec_time_ns is not None:
        print(f"HW exec time: {res.exec_time_ns} ns")
        print(f"   mean exec time: {res.mean_exec_time_ns} ns  "
              f"(slowest core {res.max_exec_time_core_id})")
        if res.instructions_and_trace is not None:
            print("   trace:", res.instructions_and_trace[1])

    out = np.empty((B, 2 * C2, N, N), np.float32)
    for core in range(NCORES):
        b, s = core // 2, core % 2
        wlo = WS * s
        out[b, :, :, wlo:wlo + WS] = res.results[core]["out"].astype(np.float32)
    return out


# revision 32
# speedup vs baseline: 1.0100x; 1.0100x over previous
"""Trainium2 Bass kernel for the spectral-gating network (nn_DAPSO).

Model (B=4, C=256, H=W=256):
  - channels 0:128   : y_h = irfft(Gh * rfft(x, axis=H))   (per-channel gate)
  - channels 128:256 : y_w = irfft(Gw * rfft(x, axis=W))
  - gates Gh/Gw from tiny MLPs (computed on device)
  - channel attention: s = sigmoid(dw(gelu(W1 @ mean_hw(y) + b)))  -> y *= s
  - y2 = gelu(BN(lc_w @ y));  out = x + y2

Key algorithmic mapping: irfft(G*rfft(x)) along an axis of length N equals
T^T diag(ghat) T x with T the orthonormal real DFT basis (cos/sin rows), so
both branches become dense TensorE matmuls (no FFT).

Sharding: 8 cores = 4 batches x 2 w-halves. Each core computes BOTH branch
outputs for its (batch, w-half) spatial region: the H-branch needs only its
w-columns; the W-branch contracts the full W axis (its forward transform is
duplicated between the pair of cores). The only cross-core communication is
a 1KB AllReduce of the pooled channel means (one per branch).

Fully SBUF-resident: the inverse transforms use the gated spectrum slice as
the *stationary* (lhsT) matmul operand, so branch outputs land channel-major
([c, spatial]) directly in SBUF (yT_h, yT_w; 8.4MB bf16 each) and the 1x1
conv consumes them without any DRAM round-trip.

Channel attention pooling comes for free from the forward spectra:
  sum_h y_h = ghat_h[0] * sqrt(N) * u_h[k=0]        (DC row strip)
  sum_{w in Ws} y_w = sum_k sigw[k] ghat_w[k,c] S[k,c],  S = sum_h u_w

Per-core layouts (host-prepped):
  xh   (256,128,128) bf16  [h, w, c]     HC-branch input slice
  xw   (256,256,128) bf16  [w, h, c]     WC-branch input (full w)
  xres (256,256,128) bf16  [c, h, w]     residual slice
  out  (256,256,128) bf16  [c, h, w]     (host casts back to f32)
"""
import sys
import os

sys.path.insert(0, "/opt/trn_rl_repo")

import numpy as np
import ml_dtypes

import concourse.bacc as bacc
import concourse.mybir as mybir
import concourse.tile as tile
from concourse import bass_utils

F32 = mybir.dt.float32
BF16 = mybir.dt.bfloat16
AF = mybir.ActivationFunctionType
ALU = mybir.AluOpType

N = 256          # H = W
C2 = 128         # channels per branch
B = 4
NCORES = 8
WS = 128         # per-core w-slice width

_BF16_NP = ml_dtypes.bfloat16


def _dft_basis():
    """Orthonormal real DFT basis T (N, N): y = T^T diag(ghat) T x == irfft(G*rfft(x))."""
    n = np.arange(N)
    k = np.arange(1, N // 2)
    T = np.zeros((N, N), np.float64)
    T[0, :] = 1.0 / np.sqrt(N)
    T[1:N // 2, :] = np.sqrt(2.0 / N) * np.cos(2 * np.pi * k[:, None] * n[None, :] / N)
    T[N // 2, :] = (1.0 / np.sqrt(N)) * ((-1.0) ** n)
    T[N // 2 + 1:, :] = np.sqrt(2.0 / N) * np.sin(2 * np.pi * k[:, None] * n[None, :] / N)
    return T.astype(np.float32)


def _part_major(a):
    """(256, ...) -> (128, 2, ...) partition-major layout."""
    a = np.asarray(a)
    return np.ascontiguousarray(a.reshape(2, 128, *a.shape[1:]).transpose(
        (1, 0) + tuple(range(2, a.ndim + 1))))


_MLPS = ("ah", "bc1", "aw", "bc2")


def _build():
    nc = bacc.Bacc("TRN2", target_bir_lowering=False, num_devices=NCORES)

    # ---------------- I/O declarations ----------------
    xh_d = nc.dram_tensor("xh", [256, 128, 128], BF16, kind="ExternalInput")
    xw_d = nc.dram_tensor("xw", [256, 256, 128], BF16, kind="ExternalInput")
    xres_d = nc.dram_tensor("xres", [256, 256, 128], BF16, kind="ExternalInput")
    tfwd_d = nc.dram_tensor("tfwd", [128, 2, 256], BF16, kind="ExternalInput")
    tinv_d = nc.dram_tensor("tinv", [128, 2, 256], BF16, kind="ExternalInput")
    tinvw_d = nc.dram_tensor("tinvw", [128, 2, 128], BF16, kind="ExternalInput")
    sigw_d = nc.dram_tensor("sigw", [128, 2], F32, kind="ExternalInput")
    omega_d = nc.dram_tensor("omega", [1, 129], F32, kind="ExternalInput")
    lam_d = nc.dram_tensor("lam", [1, 128], F32, kind="ExternalInput")
    mlp_d = {}
    for m in _MLPS:
        mlp_d[m] = dict(
            w1t=nc.dram_tensor(f"{m}_w1t", [1, 64], F32, kind="ExternalInput"),
            b1=nc.dram_tensor(f"{m}_b1v", [64, 1], F32, kind="ExternalInput"),
            w2t=nc.dram_tensor(f"{m}_w2t", [64, 64], F32, kind="ExternalInput"),
            b2=nc.dram_tensor(f"{m}_b2v", [64, 1], F32, kind="ExternalInput"),
            w3t=nc.dram_tensor(f"{m}_w3t", [64, 8], F32, kind="ExternalInput"),
            b3=nc.dram_tensor(f"{m}_b3v", [8, 1], F32, kind="ExternalInput"),
        )
    caw1t_d = nc.dram_tensor("caw1t", [128, 2, 256], F32, kind="ExternalInput")
    cab1_d = nc.dram_tensor("cab1", [128, 2], F32, kind="ExternalInput")
    dwc_d = nc.dram_tensor("dwc", [128, 2], F32, kind="ExternalInput")
    dwb_d = nc.dram_tensor("dwb", [128, 2], F32, kind="ExternalInput")
    lcwt_d = nc.dram_tensor("lcwt", [128, 2, 256], F32, kind="ExternalInput")
    bng_d = nc.dram_tensor("bng", [128, 2], F32, kind="ExternalInput")
    bnb_d = nc.dram_tensor("bnb", [128, 2], F32, kind="ExternalInput")
    bnm_d = nc.dram_tensor("bnm", [128, 2], F32, kind="ExternalInput")
    bnv_d = nc.dram_tensor("bnv", [128, 2], F32, kind="ExternalInput")

    out_d = nc.dram_tensor("out", [256, 256, 128], BF16, kind="ExternalOutput")

    arh_in = nc.dram_tensor("arh_in", [128, 1], F32)
    arh_out = nc.dram_tensor("arh_out", [128, 1], F32)
    arw_in = nc.dram_tensor("arw_in", [128, 1], F32)
    arw_out = nc.dram_tensor("arw_out", [128, 1], F32)

    with tile.TileContext(nc) as tc:
        with tc.tile_pool(name="consts", bufs=1) as consts, \
             tc.tile_pool(name="yres", bufs=1) as yres, \
             tc.tile_pool(name="xin", bufs=2) as xin, \
             tc.tile_pool(name="uch", bufs=2) as uch, \
             tc.tile_pool(name="crhs", bufs=2) as crhs, \
             tc.tile_pool(name="outp", bufs=2) as outp, \
             tc.tile_pool(name="gsb", bufs=1) as gsb, \
             tc.tile_pool(name="ps", bufs=1, space="PSUM") as ps:

            # ---------------- const loads ----------------
            # MLP weights FIRST: the gate matmuls lead the in-order Tensor
            # stream, so their weights must not queue behind bulk loads.
            mlp_t = {}
            for m in _MLPS:
                d = mlp_d[m]
                mlp_t[m] = {}
                for nm, shp in (("w1t", [1, 64]), ("b1", [64, 1]),
                                ("w2t", [64, 64]), ("b2", [64, 1]),
                                ("w3t", [64, 8]), ("b3", [8, 1])):
                    t = consts.tile(shp, F32, tag=f"{m}_{nm}")
                    nc.sync.dma_start(out=t, in_=d[nm][:])
                    mlp_t[m][nm] = t
            omega_t = consts.tile([1, 129], F32, tag="omega")
            nc.sync.dma_start(out=omega_t, in_=omega_d[:])
            lam_t = consts.tile([1, 128], F32, tag="lam")
            nc.sync.dma_start(out=lam_t, in_=lam_d[:])
            tfwd_t = consts.tile([128, 2, 256], BF16, tag="tfwd")
            nc.sync.dma_start(out=tfwd_t, in_=tfwd_d[:])
            tinv_t = consts.tile([128, 2, 256], BF16, tag="tinv")
            nc.sync.dma_start(out=tinv_t, in_=tinv_d[:])
            tinvw_t = consts.tile([128, 2, 128], BF16, tag="tinvw")
            nc.sync.dma_start(out=tinvw_t, in_=tinvw_d[:])
            sigw_t = consts.tile([128, 2], F32, tag="sigw")
            nc.sync.dma_start(out=sigw_t, in_=sigw_d[:])
            caw1t_t = consts.tile([128, 2, 256], F32, tag="caw1t")
            nc.sync.dma_start(out=caw1t_t, in_=caw1t_d[:])
            lcwt_t = consts.tile([128, 2, 256], F32, tag="lcwt")
            nc.sync.dma_start(out=lcwt_t, in_=lcwt_d[:])
            vec_t = {}
            for nm, d in (("cab1", cab1_d), ("dwc", dwc_d), ("dwb", dwb_d),
                          ("bng", bng_d), ("bnb", bnb_d), ("bnm", bnm_d), ("bnv", bnv_d)):
                vt = consts.tile([128, 2], F32, tag=f"v_{nm}")
                nc.sync.dma_start(out=vt, in_=d[:])
                vec_t[nm] = vt
            one1_t = consts.tile([1, 1], F32, tag="one1")
            nc.vector.memset(one1_t, 1.0)

            # ---------------- gate MLPs (tiny), stage-batched ----------------
            # All four heads advance layer-by-layer so each activation table
            # (Gelu) loads once per sweep instead of thrashing per-head.
            heads = (("aw", omega_t, 129), ("bc2", lam_t, 128),
                     ("ah", omega_t, 129), ("bc1", lam_t, 128))
            ptags = ("A00", "B0", "A01", "B1")
            p1 = {}
            for i, (m, xv, nk) in enumerate(heads):
                p = ps.tile([64, nk], F32, tag=ptags[i])
                nc.tensor.matmul(p, lhsT=mlp_t[m]["w1t"], rhs=xv, start=True, stop=True)
                p1[m] = p
            h1 = {}
            for i, (m, xv, nk) in enumerate(heads):
                h = gsb.tile([64, nk], F32, tag=f"m_h1_{i}")
                nc.scalar.activation(h, p1[m], AF.Gelu, bias=mlp_t[m]["b1"])
                h1[m] = h
            p2 = {}
            for i, (m, xv, nk) in enumerate(heads):
                p = ps.tile([64, nk], F32, tag=ptags[i])
                nc.tensor.matmul(p, lhsT=mlp_t[m]["w2t"], rhs=h1[m], start=True, stop=True)
                p2[m] = p
            h2 = {}
            for i, (m, xv, nk) in enumerate(heads):
                h = gsb.tile([64, nk], F32, tag=f"m_h1_{i}")
                nc.scalar.activation(h, p2[m], AF.Gelu, bias=mlp_t[m]["b2"])
                h2[m] = h
            at = {}
            for i, (m, xv, nk) in enumerate(heads):
                p = ps.tile([8, nk], F32, tag=ptags[i])
                nc.tensor.matmul(p, lhsT=mlp_t[m]["w3t"], rhs=h2[m], start=True, stop=True)
                a = gsb.tile([8, nk], BF16, tag=f"m_at{i}")
                nc.scalar.activation(a, p, AF.Identity, bias=mlp_t[m]["b3"])
                at[m] = a

            ghh = consts.tile([128, 2, 128], F32, tag="ghh")
            ghw = consts.tile([128, 2, 128], F32, tag="ghw")
            ghhb = consts.tile([128, 2, 128], BF16, tag="ghhb")
            ghwb = consts.tile([128, 2, 128], BF16, tag="ghwb")
            # transposed DC-row gate column for pool_h: G^T[:, 0] pre-softplus
            g0c_ps = ps.tile([128, 1], F32, tag="B2")
            nc.tensor.matmul(g0c_ps, lhsT=at["bc1"], rhs=at["ah"][:, 0:1],
                             start=True, stop=True)
            g0col = gsb.tile([128, 1], F32, tag="g0col")
            nc.scalar.activation(g0col, g0c_ps, AF.Sigmoid, scale=-1.0)
            nc.scalar.activation(g0col, g0col, AF.Ln)
            nc.vector.tensor_scalar_mul(g0col, g0col, float(-(8.0 ** -0.5)))

            gp = {}
            gtags = {("h", 0): "A00", ("h", 1): "A01", ("w", 0): "B0", ("w", 1): "B1"}
            for (am, bm, nmk) in (("aw", "bc2", "w"), ("ah", "bc1", "h")):
                g0 = ps.tile([128, 128], F32, tag=gtags[(nmk, 0)])
                nc.tensor.matmul(g0, lhsT=at[am][:, 0:128], rhs=at[bm], start=True, stop=True)
                gn = ps.tile([1, 128], F32, tag=gtags[(nmk, 1)])
                nc.tensor.matmul(gn, lhsT=at[am][:, 128:129], rhs=at[bm], start=True, stop=True)
                gp[(nmk, 0)] = g0
                gp[(nmk, 1)] = gn
            # softplus(z) = -ln(sigmoid(-z)); the -1 and 1/sqrt(R) fold into
            # the final scale pass.
            keys = list(gp.keys())
            sp = {}
            for i, key in enumerate(keys):
                npart = 128 if key[1] == 0 else 1
                sg = gsb.tile([128, 128], F32, tag=f"sp_sg{i}")
                nc.scalar.activation(sg[:npart, :], gp[key], AF.Sigmoid, scale=-1.0)
                sp[key] = sg
            for key in keys:
                gh = ghh if key[0] == "h" else ghw
                if key[1] == 0:
                    nc.scalar.activation(gh[:, 0, :], sp[key][:128, :], AF.Ln)
                else:
                    nc.scalar.activation(sp[key][0:1, :], sp[key][0:1, :], AF.Ln)
            for key in keys:
                gh = ghh if key[0] == "h" else ghw
                if key[1] == 1:
                    # rows 128+j of ghat equal G[j]: copy the aligned block,
                    # then overwrite row 0 with the Nyquist G[128].
                    nc.vector.tensor_copy(gh[:, 1, :], gh[:, 0, :])
                    nc.vector.tensor_copy(gh[0:1, 1, :], sp[key][0:1, :])
            for gh, ghb in ((ghh, ghhb), (ghw, ghwb)):
                nc.vector.tensor_scalar_mul(gh[:, :, :], gh[:, :, :], float(-(8.0 ** -0.5)))
                nc.vector.tensor_copy(ghb, gh)

            # ---------------- BN prep ----------------
            bninv = consts.tile([128, 2], F32, tag="bninv")
            nc.vector.tensor_scalar_add(bninv, vec_t["bnv"], 1e-5)
            nc.scalar.activation(bninv, bninv, AF.Sqrt)
            nc.vector.reciprocal(bninv, bninv)
            nc.vector.tensor_tensor(out=bninv, in0=vec_t["bng"], in1=bninv, op=ALU.mult)
            bnbeff = consts.tile([128, 2], F32, tag="bnbeff")
            nc.vector.tensor_tensor(out=bnbeff, in0=vec_t["bnm"], in1=bninv, op=ALU.mult)
            nc.vector.tensor_tensor(out=bnbeff, in0=vec_t["bnb"], in1=bnbeff, op=ALU.subtract)

            # branch outputs, channel-major, SBUF-resident
            yT_h = yres.tile([128, 256, 128], BF16, tag="yTh")   # [c, h, w]
            yT_w = yres.tile([128, 256, 128], BF16, tag="yTw")   # [c, h, w]

            # ---------------- early pool_h from xres (already c-major) -------
            # pool_h[c] = Gh[0, c] * sum_{h,w} x[c, h, w]; AllReduce #1 (65us
            # latency) is issued ~40us in and hides under branch compute.
            xacc = gsb.tile([128, 128], BF16, tag="xacc")
            nc.gpsimd.memset(xacc, 0.0)
            for ci, hc in enumerate(range(0, 256, 8)):
                xt = crhs.tile([128, 8, 128], BF16, tag="xpre")
                nc.scalar.dma_start(out=xt, in_=xres_d[0:128, hc:hc + 8, :])
                t1 = gsb.tile([128, 4, 128], BF16, tag="xt1")
                nc.gpsimd.tensor_add(t1, xt[:, 0:4, :], xt[:, 4:8, :])
                t2 = gsb.tile([128, 2, 128], BF16, tag="xt2")
                nc.gpsimd.tensor_add(t2, t1[:, 0:2, :], t1[:, 2:4, :])
                nc.gpsimd.tensor_add(xacc, xacc, t2[:, 0, :])
                nc.gpsimd.tensor_add(xacc, xacc, t2[:, 1, :])
            xcol = gsb.tile([128, 1], F32, tag="xcol")
            nc.vector.tensor_reduce(out=xcol, in_=xacc,
                                    axis=mybir.AxisListType.X, op=ALU.add)
            poolh_sb = gsb.tile([128, 1], F32, tag="poolh")
            nc.vector.tensor_tensor(out=poolh_sb, in0=g0col, in1=xcol, op=ALU.mult)
            nc.sync.dma_start(out=arh_in[:], in_=poolh_sb)
            nc.gpsimd.collective_compute(
                "AllReduce", ALU.add,
                replica_groups=[[0, 1], [2, 3], [4, 5], [6, 7]],
                ins=[arh_in[:]], outs=[arh_out[:]])

            wacc = gsb.tile([128, 32], F32, tag="wacc")

            # ---------------- branches, interleaved ----------------
            # WC chunks are Vector-heavy (gate-mults), HC chunks are
            # scatter-heavy (Scalar/GpSimd); interleaving (W,W,H) lets idle
            # engines absorb each other's load. WC finishes 2/3 in so its
            # AllReduce still hides.
            def wc_chunk(h0):
                xw_t = []
                for wt in (0, 1):
                    xt = xin.tile([128, 8, 128], BF16, tag=f"xb{wt}")
                    nc.sync.dma_start(out=xt, in_=xw_d[wt * 128:(wt + 1) * 128,
                                                      h0:h0 + 8, :])
                    xw_t.append(xt)
                ug = {}
                for kt in (0, 1):
                    for hf in (0, 1):
                        pk = ps.tile([128, 4, 128], F32, tag=f"A{kt}{hf}")
                        for wt in (0, 1):
                            nc.tensor.matmul(pk,
                                             lhsT=tfwd_t[:, wt, kt * 128:(kt + 1) * 128],
                                             rhs=xw_t[wt][:, hf * 4:hf * 4 + 4, :],
                                             start=(wt == 0), stop=(wt == 1))
                        u = uch.tile([128, 4, 128], BF16, tag=f"ug{kt}{hf}")
                        nc.vector.tensor_tensor(
                            out=u, in0=pk,
                            in1=ghwb[:, kt, :].unsqueeze(1).broadcast_to([128, 4, 128]),
                            op=ALU.mult)
                        ug[(kt, hf)] = u
                for q in (0, 1):
                    py = ps.tile([128, 4, 128], F32, tag=f"B{q}")
                    for j in range(4):
                        hi = q * 4 + j
                        for kt in (0, 1):
                            nc.tensor.matmul(py[:, j, :],
                                             lhsT=ug[(kt, hi // 4)][:, hi % 4, :],
                                             rhs=tinvw_t[:, kt, :],
                                             start=(kt == 0), stop=(kt == 1))
                    nc.scalar.activation(yT_w[:, h0 + q * 4:h0 + q * 4 + 4, :], py,
                                         AF.Copy)

            def hc_chunk(w0):
                xh_t = []
                for ht in (0, 1):
                    xt = xin.tile([128, WCH, 128], BF16, tag=f"xa{ht}")
                    nc.sync.dma_start(out=xt, in_=xh_d[ht * 128:(ht + 1) * 128,
                                                      w0:w0 + WCH, :])
                    xh_t.append(xt)
                ug = {}
                for kt in (0, 1):
                    for hf in (0, 1):
                        pk = ps.tile([128, 4, 128], F32, tag=f"A{kt}{hf}")
                        for ht in (0, 1):
                            nc.tensor.matmul(pk,
                                             lhsT=tfwd_t[:, ht, kt * 128:(kt + 1) * 128],
                                             rhs=xh_t[ht][:, hf * 4:hf * 4 + 4, :],
                                             start=(ht == 0), stop=(ht == 1))
                        u = uch.tile([128, 4, 128], BF16, tag=f"uh{kt}{hf}")
                        nc.vector.tensor_tensor(
                            out=u, in0=pk,
                            in1=ghhb[:, kt, :].unsqueeze(1).broadcast_to([128, 4, 128]),
                            op=ALU.mult)
                        ug[(kt, hf)] = u
                for wi in range(WCH):
                    py = ps.tile([128, 256], F32, tag=f"B{2 + wi % 2}")
                    for kt in (0, 1):
                        nc.tensor.matmul(py, lhsT=ug[(kt, wi // 4)][:, wi % 4, :],
                                         rhs=tinv_t[:, kt, :],
                                         start=(kt == 0), stop=(kt == 1))
                    hst = uch.tile([128, 256], BF16, tag=f"hst{wi % 2}")
                    nc.scalar.activation(hst, py, AF.Copy)
                    eng = (nc.gpsimd, nc.scalar, nc.gpsimd, nc.vector,
                           nc.gpsimd, nc.scalar, nc.gpsimd, nc.scalar)[wi]
                    if eng is nc.scalar:
                        nc.scalar.activation(yT_h[:, :, w0 + wi], hst, AF.Copy)
                    else:
                        eng.tensor_copy(yT_h[:, :, w0 + wi], hst)

            def wc_slab(sl):
                red = gsb.tile([128, 32], F32, tag="wred")
                nc.vector.tensor_reduce(out=red, in_=yT_w[:, sl * 32:(sl + 1) * 32, :],
                                        axis=mybir.AxisListType.X, op=ALU.add)
                if sl == 0:
                    nc.vector.tensor_copy(wacc, red)
                else:
                    nc.vector.tensor_tensor(out=wacc, in0=wacc, in1=red, op=ALU.add)

            WCH = 8
            wc_i, hc_i = 0, 0
            for step in range(48):
                if step % 3 < 2 and wc_i < 32:
                    wc_chunk(wc_i * 8)
                    wc_i += 1
                    if wc_i % 4 == 0:
                        wc_slab(wc_i // 4 - 1)
                elif hc_i < 16:
                    hc_chunk(hc_i * 8)
                    hc_i += 1

            # pool_w[c] = sum_{h, local w} y_w[c, h, w]  -> AllReduce #2
            poolw_sb = gsb.tile([128, 1], F32, tag="poolw")
            nc.vector.tensor_reduce(out=poolw_sb, in_=wacc,
                                    axis=mybir.AxisListType.X, op=ALU.add)
            nc.sync.dma_start(out=arw_in[:], in_=poolw_sb)
            nc.gpsimd.collective_compute(
                "AllReduce", ALU.add,
                replica_groups=[[0, 1], [2, 3], [4, 5], [6, 7]],
                ins=[arw_in[:]], outs=[arw_out[:]])

            p_sb = []
            for ct, aro in ((0, arh_out), (1, arw_out)):
                pt = gsb.tile([128, 1], F32, tag=f"p_ar{ct}")
                nc.gpsimd.dma_start(out=pt, in_=aro[:])
                p_sb.append(pt)

            # ---------------- channel attention -> folded conv weights ----------------
            q_sb = []
            for ot in (0, 1):
                q_ps = ps.tile([128, 1], F32, tag=f"B{ot}")
                for ct in (0, 1):
                    nc.tensor.matmul(q_ps, lhsT=caw1t_t[:, ct, ot * 128:(ot + 1) * 128],
                                     rhs=p_sb[ct], start=(ct == 0), stop=(ct == 1))
                qt = gsb.tile([128, 1], F32, tag=f"q{ot}")
                nc.scalar.activation(qt, q_ps, AF.Gelu, bias=vec_t["cab1"][:, ot:ot + 1])
                nc.vector.tensor_tensor(out=qt, in0=qt, in1=vec_t["dwc"][:, ot:ot + 1],
                                        op=ALU.mult)
                q_sb.append(qt)
            s_sb = []
            for ot in (0, 1):
                s_t = gsb.tile([128, 1], F32, tag=f"s{ot}")
                nc.scalar.activation(s_t, q_sb[ot], AF.Sigmoid, bias=vec_t["dwb"][:, ot:ot + 1])
                s_sb.append(s_t)
            wsc = consts.tile([128, 2, 256], BF16, tag="wsc")
            for ct in (0, 1):
                nc.vector.tensor_scalar_mul(wsc[:, ct, :], lcwt_t[:, ct, :], s_sb[ct])

            # ---------------- conv 1x1 + BN + GELU + residual add ----------------
            HCH = 8
            for h0 in range(0, 256, HCH):
                xts = []
                for ot in (0, 1):
                    xt = crhs.tile([128, HCH, 128], BF16, tag=f"xr{ot}")
                    nc.scalar.dma_start(out=xt, in_=xres_d[ot * 128:(ot + 1) * 128,
                                                           h0:h0 + HCH, :])
                    xts.append(xt)
                for ot in (0, 1):
                    gstg = outp.tile([128, HCH, 128], BF16, tag=f"gstg{ot}")
                    for sl in (0, 4):
                        rh = yT_h[:, h0 + sl:h0 + sl + 4, :]
                        rw = yT_w[:, h0 + sl:h0 + sl + 4, :]
                        po = ps.tile([128, 4, 128], F32, tag=f"A{ot}{sl // 4}")
                        nc.tensor.matmul(po, lhsT=wsc[:, 0, ot * 128:(ot + 1) * 128],
                                         rhs=rh, start=True, stop=False)
                        nc.tensor.matmul(po, lhsT=wsc[:, 1, ot * 128:(ot + 1) * 128],
                                         rhs=rw, start=False, stop=True)
                        nc.scalar.activation(gstg[:, sl:sl + 4, :], po, AF.Gelu,
                                             bias=bnbeff[:, ot:ot + 1],
                                             scale=bninv[:, ot:ot + 1])
                    nc.vector.tensor_tensor(out=gstg, in0=gstg, in1=xts[ot],
                                            op=ALU.add)
                    nc.sync.dma_start(out=out_d[ot * 128:(ot + 1) * 128, h0:h0 + HCH, :],
                                        in_=gstg)

    nc.compile()
    return nc


_NC_CACHE = None


def _get_nc():
    global _NC_CACHE
    if _NC_CACHE is None:
        _NC_CACHE = _build()
    return _NC_CACHE


def _host_consts(inputs, core):
    """Per-core constant inputs (everything except the x shards)."""
    s = core % 2
    wlo = WS * s
    T = _dft_basis()
    d = {}
    d["tfwd"] = _part_major(np.ascontiguousarray(T.T)).astype(_BF16_NP)
    d["tinv"] = _part_major(T).astype(_BF16_NP)
    d["tinvw"] = _part_major(np.ascontiguousarray(T[:, wlo:wlo + WS])).astype(_BF16_NP)
    d["sigw"] = _part_major(T[:, wlo:wlo + WS].sum(axis=1)).astype(np.float32)
    d["omega"] = (np.arange(129, dtype=np.float32) / 128.0 - 1.0).reshape(1, 129)
    d["lam"] = np.linspace(-1.0, 1.0, 128, dtype=np.float32).reshape(1, 128)
    for m in _MLPS:
        d[f"{m}_w1t"] = np.ascontiguousarray(inputs[f"{m}_w1"].T).astype(np.float32)
        d[f"{m}_b1v"] = inputs[f"{m}_b1"].reshape(64, 1).astype(np.float32)
        d[f"{m}_w2t"] = np.ascontiguousarray(inputs[f"{m}_w2"].T).astype(np.float32)
        d[f"{m}_b2v"] = inputs[f"{m}_b2"].reshape(64, 1).astype(np.float32)
        d[f"{m}_w3t"] = np.ascontiguousarray(inputs[f"{m}_w3"].T).astype(np.float32)
        d[f"{m}_b3v"] = inputs[f"{m}_b3"].reshape(8, 1).astype(np.float32)
    d["caw1t"] = _part_major(np.ascontiguousarray(inputs["ca_w1"].T) / 65536.0).astype(np.float32)
    d["cab1"] = _part_major(inputs["ca_b1"]).astype(np.float32)
    d["dwc"] = _part_major(np.ascontiguousarray(inputs["ca_dw"][:, 1, 1])).astype(np.float32)
    d["dwb"] = _part_major(inputs["ca_db"]).astype(np.float32)
    d["lcwt"] = _part_major(np.ascontiguousarray(inputs["lc_w"].T)).astype(np.float32)
    d["bng"] = _part_major(inputs["bn_g"]).astype(np.float32)
    d["bnb"] = _part_major(inputs["bn_b"]).astype(np.float32)
    d["bnm"] = _part_major(inputs["bn_m"]).astype(np.float32)
    d["bnv"] = _part_major(inputs["bn_v"]).astype(np.float32)
    return d


def kernel(**inputs):
    x = np.asarray(inputs["x"], np.float32)
    nc = _get_nc()

    in_maps = []
    for core in range(NCORES):
        b, s = core // 2, core % 2
        wlo = WS * s
        m = _host_consts(inputs, core)
        m["xh"] = np.ascontiguousarray(
            x[b, :C2, :, wlo:wlo + WS].transpose(1, 2, 0)).astype(_BF16_NP)
        m["xw"] = np.ascontiguousarray(
            x[b, C2:, :, :].transpose(2, 1, 0)).astype(_BF16_NP)
        m["xres"] = np.ascontiguousarray(x[b, :, :, wlo:wlo + WS]).astype(_BF16_NP)
        in_maps.append(m)

    trace = os.environ.get("BASS_KERNEL_TRACE", "0") == "1"
    res = bass_utils.run_bass_kernel_spmd(
        nc, in_maps, core_ids=list(range(NCORES)),
        trace=trace, trace_cores=list(range(NCORES)) if trace else None,
        stitch_traces=False)
    if trace and res.exec_time_ns is not None:
        print(f"HW exec time: {res.exec_time_ns} ns")
        print(f"   mean exec time: {res.mean_exec_time_ns} ns  "
              f"(slowest core {res.max_exec_time_core_id})")
        if res.instructions_and_trace is not None:
            print("   trace:", res.instructions_and_trace[1])

    out = np.empty((B, 2 * C2, N, N), np.float32)
    for core in range(NCORES):
        b, s = core // 2, core % 2
        wlo = WS * s
        out[b, :, :, wlo:wlo + WS] = res.results[core]["out"].astype(np.float32)
    return out
